# revision 33
# baseline (speedup 1.0000x reference)
"""MoE FFN (EnterpriseFFN) Trainium2 kernel.

8192 tokens x d_model=1024, 8 experts (hidden 512), top-2 gating where every
selected expert is scaled by the SUM of the top-2 softmax gates.

Distribution: data-parallel over tokens -- each of the 8 NeuronCores runs
1024 tokens through all 8 experts (dense compute, masked combine, exactly
like the reference einsum formulation). Expert weights are replicated.

Per-core pipeline (activations kept transposed, [feature, token]):
  1. Load x [1024 tok, 1024 d]; PE-transpose to fp32 xg (gating) and bf16 xT
     (FFN) tiles, with per-chunk gating (softmax + top-2 via max / masked-max
     on DVE, exact fp32 logits so the top-2 selection matches the oracle);
     S[tok, e] = sel * tok_w is PE-transposed to ST [e, tok]. Expert 0's
     layer 1 is interleaved to keep the PE stream dense (HAM warm).
  2. Per expert e: hT = gelu(w1[e].T-chunks @ xT + b1) on PE/ACT (bf16 in,
     fp32 PSUM), scaled along tokens by a ones-matmul broadcast of ST's row;
     layer 2 accumulates expert PAIRS plus the rank-8 b2 @ S matmul in PSUM;
     a fp32 SBUF accumulator sums the pairs.
  3. Store yT [d, tok]; the host transposes shards back and concatenates.

FFN matmuls run in bf16 (fast weight load, 1 cyc/row); gating runs in exact
fp32. Weight tiles are DMA-staged fp32 then cast to bf16 on ACT/DVE.
"""

import numpy as np

import bass_rust
import concourse.bass as bass
import concourse.tile as tile
from concourse import mybir
from concourse.bass_utils import run_bass_kernel_spmd
from concourse.masks import make_identity
from concourse.tile_rust import add_dep_helper

N_CORES = 8
B, S, D, H, E = 4, 2048, 1024, 512, 8
NTOK = B * S          # 8192 total tokens
TOK = NTOK // N_CORES  # 1024 tokens per core
KD = D // 128          # 8 d_model chunks
KH = H // 128          # 4 hidden chunks
TT = TOK // 128        # 8 token chunks
NF = 512               # matmul moving free width
NHF = TOK // NF        # 2 token halves

FP = mybir.dt.float32
BF = mybir.dt.bfloat16
AF = mybir.ActivationFunctionType
ALU = mybir.AluOpType
AX = mybir.AxisListType


def _legalize_sync_waits(nc, max_waits=1):
    """Split multi-wait instructions for this walrus (1 sync wait per inst).

    Any instruction carrying more than ``max_waits`` sync-wait commands gets
    the extra waits peeled onto same-engine NoOps inserted immediately before
    it -- identical semantics (engine program order), legal ISA encoding.
    """
    n_split = 0
    for f in nc.m.functions:
        for bb in f.blocks:
            new_insts = []
            for inst in bb.instructions:
                si = getattr(inst, "sync_info", None)
                if si is not None and len(si.on_wait) > max_waits:
                    waits = list(si.on_wait)
                    for w in waits[max_waits:]:
                        nop = mybir.InstNoOp(
                            name=nc.get_next_instruction_name(), ins=[], outs=[]
                        )
                        nop.engine = inst.engine
                        nop.sync_info = bass_rust.SyncInfo(
                            on_wait=[w], on_update=[]
                        )
                        new_insts.append(nop)
                        n_split += 1
                    inst.sync_info = bass_rust.SyncInfo(
                        on_wait=waits[:max_waits], on_update=list(si.on_update)
                    )
                new_insts.append(inst)
            bb.instructions = new_insts
    return n_split


def _emit(tc, xt32, xt16, gw, w1, b1, w2, b2, outT):
    nc = tc.nc

    with (
        tc.tile_pool(name="const", bufs=1) as const_pool,
        tc.tile_pool(name="persist", bufs=1) as persist,
        tc.tile_pool(name="w1pool", bufs=3) as w1pool,
        tc.tile_pool(name="w2pool", bufs=3) as w2pool,
        tc.tile_pool(name="bpool", bufs=4) as bpool,
        tc.tile_pool(name="hpool", bufs=3) as hpool,
        tc.tile_pool(name="sbpool", bufs=3) as sbpool,
        tc.tile_pool(name="fpsum", bufs=4, space="PSUM") as fpsum,
    ):
        ident = const_pool.tile([128, 128], FP, tag="ident")
        make_identity(nc, ident[:])
        ones_f = const_pool.tile([1, 128], FP, tag="ones_f")
        nc.vector.memset(ones_f[:], 1.0)
        ones_row = const_pool.tile([1, 128], BF, tag="ones")
        nc.vector.tensor_copy(ones_row[:], ones_f[:])

        # gate_w [D, E] -> per-d-chunk [128, E] blocks, free-concatenated
        gw_sb = const_pool.tile([128, KD * E], FP, tag="gw")
        for k in range(KD):
            nc.sync.dma_start(
                gw_sb[:, k * E:(k + 1) * E], gw[k * 128:(k + 1) * 128, :]
            )
        # b2 [E, D] natural layout (E on partitions), cast to bf16
        b2f = const_pool.tile([E, D], FP, tag="b2f")
        nc.gpsimd.dma_start(b2f[:], b2[:, :])
        b2T = persist.tile([E, D], BF, tag="b2T")
        nc.vector.tensor_copy(b2T[:], b2f[:])

        # bf16 xT for FFN matmuls; exact fp32 xg (stage-scoped) for gating so
        # the top-2 selection matches the oracle.
        xT = [
            persist.tile([128, TOK], BF, tag=f"xT{d}", name=f"xT{d}")
            for d in range(KD)
        ]
        ST = persist.tile([E, TOK], BF, tag="ST")
        acc = [
            persist.tile([128, TOK], FP, tag=f"acc{m}", name=f"acc{m}")
            for m in range(KD)
        ]

        # weights arrive bf16 pre-laid-out from the host ([E, 128, X] SBUF
        # image, fully contiguous) -- one wide DMA per expert half, split
        # over gpsimd SW-DGE + sync/scalar HW queues.
        loaded = {}
        _wprev = [None]

        def _wdma(dst, srcap, after=None):
            di = nc.gpsimd.dma_start(dst, srcap)
            prev = after if after is not None else _wprev[0]
            if prev is not None:
                add_dep_helper(di.ins, prev, reason="ring order")
            _wprev[0] = di.ins
            return di

        def _load_w1(e, after=None):
            w1t = w1pool.tile([128, KD * H], BF, tag="w1", name="w1t")
            _wdma(w1t[:], w1[e], after=after)
            b1t = bpool.tile([128, KH], FP, tag="b1", name="b1t")
            nc.sync.dma_start(b1t[:], b1[e])
            loaded[e] = (w1t, b1t)

        def _load_w2(e, after=None):
            w2t = w2pool.tile([128, KH * D], BF, tag="w2", name="w2t")
            _wdma(w2t[:], w2[e])
            loaded_w2[e] = w2t

        def _l1_half(w1t, b1t, hts, hf, sbt=None):
            # layer 1 for one token half: hts[:, mh, hf] = gelu(w1.T @ xT + b1)
            # scaled by the expert's per-token gate weight when sbt is given
            for mh in range(KH):
                ph = fpsum.tile([128, NF], FP, tag="ph", name="ph")
                for kd in range(KD):
                    nc.tensor.matmul(
                        ph[:],
                        w1t[:, kd * H + mh * 128:kd * H + (mh + 1) * 128],
                        xT[kd][:, hf * NF:(hf + 1) * NF],
                        start=(kd == 0),
                        stop=(kd == KD - 1),
                    )
                hsl = hts[:, mh * TOK + hf * NF:mh * TOK + (hf + 1) * NF]
                nc.scalar.activation(hsl, ph[:], AF.Gelu, bias=b1t[:, mh:mh + 1])
                if sbt is not None:
                    nc.vector.tensor_tensor(
                        hsl, hsl, sbt[:, hf * NF:(hf + 1) * NF], op=ALU.mult
                    )

        loaded_w2 = {}
        xlast = {}
        hts_pair = {}
        w2_pair = {}

        # ---- stage 1: x load + transpose + gating, with expert-0 layer 1
        # interleaved so the PE stream stays dense (HAM warm) ---------------
        with (
            tc.tile_pool(name="xg", bufs=1) as xg_pool,
            tc.tile_pool(name="gpsum", bufs=2, space="PSUM") as gpsum,
            tc.tile_pool(name="gtmp", bufs=3) as gtmp,
        ):
            xg = [
                xg_pool.tile([128, TOK], FP, tag=f"xg{d}", name=f"xg{d}")
                for d in range(KD)
            ]

            # x host-transposed. fp32 (gating) streams on the gpsimd ring in
            # gating-consumption order with w1[0] interleaved at its use
            # point; bf16 xT rides the otherwise-idle sync/scalar HW queues.
            xq = [(0, 256), (256, 512), (512, 768), (768, 1024)]
            for qi, (c0, c1) in enumerate(xq):
                qs = slice(c0, c1)
                for dd in range(KD):
                    di = nc.gpsimd.dma_start(
                        xg[dd][:, qs], xt32[dd * 128:(dd + 1) * 128, qs]
                    )
                    xlast[("f", qi)] = di.ins
            for half in range(2):
                hs = slice(half * 512, (half + 1) * 512)
                for dd in range(KD):
                    eng = nc.sync if dd % 2 == 0 else nc.scalar
                    di = eng.dma_start(
                        xT[dd][:, hs], xt16[dd * 128:(dd + 1) * 128, hs]
                    )
                    xlast[("b", half)] = di.ins

            def _tchunk(t):
                # gating for this token chunk (exact fp32)
                ts = slice(t * 128, (t + 1) * 128)
                pg = gpsum.tile([128, E], FP, tag="pg", name="pg")
                for d in range(KD):
                    nc.tensor.matmul(
                        pg[:],
                        xg[d][:, ts],
                        gw_sb[:, d * E:(d + 1) * E],
                        start=(d == 0),
                        stop=(d == KD - 1),
                    )
                m = gtmp.tile([128, 1], FP, tag="m", name="m")
                nc.vector.tensor_reduce(m[:], pg[:], axis=AX.X, op=ALU.max)
                nm = gtmp.tile([128, 1], FP, tag="nm", name="nm")
                nc.vector.tensor_scalar(nm[:], m[:], -1.0, None, op0=ALU.mult)
                ex = gtmp.tile([128, E], FP, tag="ex", name="ex")
                nc.scalar.activation(ex[:], pg[:], AF.Exp, bias=nm[:, 0:1])
                ssum = gtmp.tile([128, 1], FP, tag="ssum", name="ssum")
                nc.vector.tensor_reduce(ssum[:], ex[:], axis=AX.X, op=ALU.add)
                r = gtmp.tile([128, 1], FP, tag="r", name="r")
                nc.vector.reciprocal(r[:], ssum[:])
                g = gtmp.tile([128, E], FP, tag="g", name="g")
                nc.vector.tensor_scalar(g[:], ex[:], r[:, 0:1], None, op0=ALU.mult)
                # top-2: m1 = max, m2 = max after suppressing the argmax
                m1 = gtmp.tile([128, 1], FP, tag="m1", name="m1")
                nc.vector.tensor_reduce(m1[:], g[:], axis=AX.X, op=ALU.max)
                is1 = gtmp.tile([128, E], FP, tag="is1", name="is1")
                nc.vector.tensor_scalar(
                    is1[:], g[:], m1[:, 0:1], None, op0=ALU.is_ge
                )
                g2 = gtmp.tile([128, E], FP, tag="g2", name="g2")
                nc.vector.tensor_scalar(g2[:], is1[:], -2.0, None, op0=ALU.mult)
                nc.vector.tensor_tensor(g2[:], g2[:], g[:], op=ALU.add)
                m2 = gtmp.tile([128, 1], FP, tag="m2", name="m2")
                nc.vector.tensor_reduce(m2[:], g2[:], axis=AX.X, op=ALU.max)
                tokw = gtmp.tile([128, 1], FP, tag="tokw", name="tokw")
                nc.vector.tensor_tensor(tokw[:], m1[:], m2[:], op=ALU.add)
                sel = gtmp.tile([128, E], FP, tag="sel", name="sel")
                nc.vector.tensor_scalar(
                    sel[:], g[:], m2[:, 0:1], None, op0=ALU.is_ge
                )
                sw = gtmp.tile([128, E], FP, tag="sw", name="sw")
                nc.vector.tensor_scalar(
                    sw[:], sel[:], tokw[:, 0:1], None, op0=ALU.mult
                )
                # transpose S chunk [128, E] -> ST[:, t*128:+128] (bf16)
                pst = gpsum.tile([128, 128], FP, tag="pst", name="pst")
                nc.tensor.transpose(pst[0:E, :], sw[:], ident[:])
                nc.vector.tensor_copy(ST[:, ts], pst[0:E, :])

            # all 8 gating chunks run back-to-back (the ring streams fp32 x
            # in the same order); expert 0's layer 1 follows once w1[0]
            # lands on the ring right after the gating stream
            for t in range(TT):
                _tchunk(t)
            _load_w1(0, after=xlast[("f", 3)])
            hts0 = hpool.tile([128, KH * TOK], BF, tag="h", name="hts0")
            hts_pair[0] = hts0
            _l1_half(loaded[0][0], loaded[0][1], hts0, 0)
            _load_w1(1)
            _l1_half(loaded[0][0], loaded[0][1], hts0, 1)

        # ---- stage 2: per-expert FFN (bf16), expert-pair PSUM accum -------
        with (
            tc.tile_pool(name="bpsum", bufs=1, space="PSUM") as bpsum,
            tc.tile_pool(name="ypsum", bufs=3, space="PSUM") as ypsum,
        ):
            def _sbt_for(e):
                # expert's S row to partition 0, then broadcast to all 128
                # partitions via a K=1 ones-matmul
                ste = sbpool.tile([1, TOK], BF, tag="ste", name="ste")
                nc.sync.dma_start(ste[:], ST[e:e + 1, :])
                sbt = sbpool.tile([128, TOK], BF, tag="sb", name="sbt")
                for hf in range(NHF):
                    pb = bpsum.tile([128, NF], FP, tag="pb", name="pb")
                    nc.tensor.matmul(
                        pb[:],
                        ones_row[:],
                        ste[:, hf * NF:(hf + 1) * NF],
                        start=True,
                        stop=True,
                    )
                    nc.vector.tensor_copy(sbt[:, hf * NF:(hf + 1) * NF], pb[:])
                return sbt

            def _scale(hts, sbt):
                for mh in range(KH):
                    for hf in range(NHF):
                        hsl = hts[
                            :, mh * TOK + hf * NF:mh * TOK + (hf + 1) * NF
                        ]
                        nc.vector.tensor_tensor(
                            hsl, hsl, sbt[:, hf * NF:(hf + 1) * NF],
                            op=ALU.mult,
                        )

            for e in range(E):
                eo = e % 2
                # w2 of the CURRENT pair is consumed before the w1 of
                # expert e+2 -- emit in ring-consumption order
                if e == 0:
                    _load_w2(0)
                    _load_w2(1)
                elif e + 1 < E:
                    _load_w2(e + 1)
                if e + 2 < E:
                    _load_w1(e + 2)
                w1t, b1t = loaded.pop(e)
                sbt = _sbt_for(e)
                if e > 0:
                    hts = hpool.tile([128, KH * TOK], BF, tag="h", name="hts")
                    hts_pair[eo] = hts
                    _l1_half(w1t, b1t, hts, 0, sbt=sbt)
                    _l1_half(w1t, b1t, hts, 1, sbt=sbt)
                else:
                    _scale(hts_pair[eo], sbt)
                w2_pair[eo] = loaded_w2.pop(e)
                if eo == 0:
                    continue
                # layer 2 for the expert pair (e-1, e), PSUM-accumulated
                for md in range(KD):
                    for hf in range(NHF):
                        py = ypsum.tile([128, NF], FP, tag="py", name="py")
                        for po in (0, 1):
                            for kh in range(KH):
                                nc.tensor.matmul(
                                    py[:],
                                    w2_pair[po][
                                        :,
                                        kh * D + md * 128:kh * D + (md + 1) * 128,
                                    ],
                                    hts_pair[po][
                                        :,
                                        kh * TOK + hf * NF:kh * TOK + (hf + 1) * NF,
                                    ],
                                    start=(po == 0 and kh == 0),
                                    stop=(po == 1 and kh == KH - 1 and e != 1),
                                )
                        if e == 1:
                            # + sum_e S_e[tok] * b2[e, d] as a rank-8 matmul
                            nc.tensor.matmul(
                                py[:],
                                b2T[:, md * 128:(md + 1) * 128],
                                ST[:, hf * NF:(hf + 1) * NF],
                                start=False,
                                stop=True,
                            )
                        asl = acc[md][:, hf * NF:(hf + 1) * NF]
                        if e == 1:
                            nc.vector.tensor_copy(asl, py[:])
                        else:
                            nc.vector.tensor_tensor(asl, asl, py[:], op=ALU.add)
                        if e == E - 1 and hf == NHF - 1:
                            # final pair: stream this d-chunk out right away
                            nc.gpsimd.dma_start(
                                outT[md * 128:(md + 1) * 128, :],
                                acc[md][:, :],
                            )


_CACHED_NC = None


def _build():
    global _CACHED_NC
    if _CACHED_NC is not None:
        return _CACHED_NC
    nc = bass.Bass(
        "TRN2", target_bir_lowering=False, debug=False, num_devices=N_CORES
    )
    xt32 = nc.dram_tensor("xt32", [D, TOK], FP, kind="ExternalInput").ap()
    xt16 = nc.dram_tensor("xt16", [D, TOK], BF, kind="ExternalInput").ap()
    gw = nc.dram_tensor("gate_w", [D, E], FP, kind="ExternalInput").ap()
    w1 = nc.dram_tensor("w1", [E, 128, KD * H], BF, kind="ExternalInput").ap()
    b1 = nc.dram_tensor("b1", [E, 128, KH], FP, kind="ExternalInput").ap()
    w2 = nc.dram_tensor("w2", [E, 128, KH * D], BF, kind="ExternalInput").ap()
    b2 = nc.dram_tensor("b2", [E, D], FP, kind="ExternalInput").ap()
    outT = nc.dram_tensor("outT", [D, TOK], FP, kind="ExternalOutput").ap()
    with tile.TileContext(nc) as tc:
        _emit(tc, xt32, xt16, gw, w1, b1, w2, b2, outT)
    _legalize_sync_waits(nc)
    _CACHED_NC = nc
    return nc


def run(inputs, **spmd_kwargs):
    """Shard, run on 8 cores, unshard. Returns (out [B,S,D], BassKernelResults)."""
    nc = _build()
    xf = np.asarray(inputs["x"], dtype=np.float32).reshape(NTOK, D)
    import ml_dtypes
    shared = {
        k: np.ascontiguousarray(np.asarray(inputs[k], dtype=np.float32))
        for k in ("gate_w", "b2")
    }
    # b1 pre-laid to the SBUF image [E, 128, KH] (the on-device rearrange
    # was a 512-way 4-byte scatter DMA that congested the sync queue)
    shared["b1"] = np.ascontiguousarray(
        np.asarray(inputs["b1"], dtype=np.float32)
        .reshape(E, KH, 128).transpose(0, 2, 1)
    )
    shared["w1"] = np.ascontiguousarray(
        np.asarray(inputs["w1"], dtype=np.float32)
        .astype(ml_dtypes.bfloat16)
        .reshape(E, KD, 128, H).transpose(0, 2, 1, 3).reshape(E, 128, KD * H)
    )
    shared["w2"] = np.ascontiguousarray(
        np.asarray(inputs["w2"], dtype=np.float32)
        .astype(ml_dtypes.bfloat16)
        .reshape(E, KH, 128, D).transpose(0, 2, 1, 3).reshape(E, 128, KH * D)
    )
    import ml_dtypes as _mld
    in_maps = []
    for c in range(N_CORES):
        shT = np.ascontiguousarray(xf[c * TOK:(c + 1) * TOK].T)
        in_maps.append({
            "xt32": shT,
            "xt16": shT.astype(_mld.bfloat16),
            **shared,
        })
    res = run_bass_kernel_spmd(nc, in_maps, list(range(N_CORES)), **spmd_kwargs)
    out = np.concatenate(
        [res.results[c]["outT"].T for c in range(N_CORES)], axis=0
    )
    return out.reshape(B, S, D).astype(np.float32, copy=False), res


def kernel(**inputs):
    out, _ = run(inputs)
    return out



# revision 34
# speedup vs baseline: 1.0004x; 1.0004x over previous
"""MoE FFN (EnterpriseFFN) Trainium2 kernel.

8192 tokens x d_model=1024, 8 experts (hidden 512), top-2 gating where every
selected expert is scaled by the SUM of the top-2 softmax gates.

Distribution: data-parallel over tokens -- each of the 8 NeuronCores runs
1024 tokens through all 8 experts (dense compute, masked combine, exactly
like the reference einsum formulation). Expert weights are replicated.

Per-core pipeline (activations kept transposed, [feature, token]):
  1. Load x [1024 tok, 1024 d]; PE-transpose to fp32 xg (gating) and bf16 xT
     (FFN) tiles, with per-chunk gating (softmax + top-2 via max / masked-max
     on DVE, exact fp32 logits so the top-2 selection matches the oracle);
     S[tok, e] = sel * tok_w is PE-transposed to ST [e, tok]. Expert 0's
     layer 1 is interleaved to keep the PE stream dense (HAM warm).
  2. Per expert e: hT = gelu(w1[e].T-chunks @ xT + b1) on PE/ACT (bf16 in,
     fp32 PSUM), scaled along tokens by a ones-matmul broadcast of ST's row;
     layer 2 accumulates expert PAIRS plus the rank-8 b2 @ S matmul in PSUM;
     a fp32 SBUF accumulator sums the pairs.
  3. Store yT [d, tok]; the host transposes shards back and concatenates.

FFN matmuls run in bf16 (fast weight load, 1 cyc/row); gating runs in exact
fp32. Weight tiles are DMA-staged fp32 then cast to bf16 on ACT/DVE.
"""

import numpy as np

import bass_rust
import concourse.bass as bass
import concourse.tile as tile
from concourse import mybir
from concourse.bass_utils import run_bass_kernel_spmd
from concourse.masks import make_identity
from concourse.tile_rust import add_dep_helper

N_CORES = 8
B, S, D, H, E = 4, 2048, 1024, 512, 8
NTOK = B * S          # 8192 total tokens
TOK = NTOK // N_CORES  # 1024 tokens per core
KD = D // 128          # 8 d_model chunks
KH = H // 128          # 4 hidden chunks
TT = TOK // 128        # 8 token chunks
NF = 512               # matmul moving free width
NHF = TOK // NF        # 2 token halves

FP = mybir.dt.float32
BF = mybir.dt.bfloat16
AF = mybir.ActivationFunctionType
ALU = mybir.AluOpType
AX = mybir.AxisListType


def _legalize_sync_waits(nc, max_waits=1):
    """Split multi-wait instructions for this walrus (1 sync wait per inst).

    Any instruction carrying more than ``max_waits`` sync-wait commands gets
    the extra waits peeled onto same-engine NoOps inserted immediately before
    it -- identical semantics (engine program order), legal ISA encoding.
    """
    n_split = 0
    for f in nc.m.functions:
        for bb in f.blocks:
            new_insts = []
            for inst in bb.instructions:
                si = getattr(inst, "sync_info", None)
                if si is not None and len(si.on_wait) > max_waits:
                    waits = list(si.on_wait)
                    for w in waits[max_waits:]:
                        nop = mybir.InstNoOp(
                            name=nc.get_next_instruction_name(), ins=[], outs=[]
                        )
                        nop.engine = inst.engine
                        nop.sync_info = bass_rust.SyncInfo(
                            on_wait=[w], on_update=[]
                        )
                        new_insts.append(nop)
                        n_split += 1
                    inst.sync_info = bass_rust.SyncInfo(
                        on_wait=waits[:max_waits], on_update=list(si.on_update)
                    )
                new_insts.append(inst)
            bb.instructions = new_insts
    return n_split


def _emit(tc, xt32, xt16, gw, w1, b1, w2, b2, outT):
    nc = tc.nc

    with (
        tc.tile_pool(name="const", bufs=1) as const_pool,
        tc.tile_pool(name="persist", bufs=1) as persist,
        tc.tile_pool(name="w1pool", bufs=3) as w1pool,
        tc.tile_pool(name="w2pool", bufs=3) as w2pool,
        tc.tile_pool(name="bpool", bufs=4) as bpool,
        tc.tile_pool(name="hpool", bufs=3) as hpool,
        tc.tile_pool(name="sbpool", bufs=3) as sbpool,
        tc.tile_pool(name="fpsum", bufs=3, space="PSUM") as fpsum,
    ):
        ident = const_pool.tile([128, 128], FP, tag="ident")
        make_identity(nc, ident[:])
        ones_f = const_pool.tile([1, 128], FP, tag="ones_f")
        nc.vector.memset(ones_f[:], 1.0)
        ones_row = const_pool.tile([1, 128], BF, tag="ones")
        nc.vector.tensor_copy(ones_row[:], ones_f[:])

        # gate_w [D, E] -> per-d-chunk [128, E] blocks, free-concatenated
        gw_sb = const_pool.tile([128, KD * E], FP, tag="gw")
        for k in range(KD):
            nc.sync.dma_start(
                gw_sb[:, k * E:(k + 1) * E], gw[k * 128:(k + 1) * 128, :]
            )
        # b2 [E, D] natural layout (E on partitions), cast to bf16
        b2f = const_pool.tile([E, D], FP, tag="b2f")
        nc.gpsimd.dma_start(b2f[:], b2[:, :])
        b2T = persist.tile([E, D], BF, tag="b2T")
        nc.vector.tensor_copy(b2T[:], b2f[:])

        # bf16 xT for FFN matmuls; exact fp32 xg (stage-scoped) for gating so
        # the top-2 selection matches the oracle.
        xT = [
            persist.tile([128, TOK], BF, tag=f"xT{d}", name=f"xT{d}")
            for d in range(KD)
        ]
        ST = persist.tile([E, TOK], BF, tag="ST")
        acc = [
            persist.tile([128, TOK], FP, tag=f"acc{m}", name=f"acc{m}")
            for m in range(KD)
        ]

        # weights arrive bf16 pre-laid-out from the host ([E, 128, X] SBUF
        # image, fully contiguous) -- one wide DMA per expert half, split
        # over gpsimd SW-DGE + sync/scalar HW queues.
        loaded = {}
        _wprev = [None]

        def _wdma(dst, srcap, after=None):
            di = nc.gpsimd.dma_start(dst, srcap)
            prev = after if after is not None else _wprev[0]
            if prev is not None:
                add_dep_helper(di.ins, prev, reason="ring order")
            _wprev[0] = di.ins
            return di

        def _load_w1(e, after=None):
            w1t = w1pool.tile([128, KD * H], BF, tag="w1", name="w1t")
            _wdma(w1t[:], w1[e], after=after)
            b1t = bpool.tile([128, KH], FP, tag="b1", name="b1t")
            nc.sync.dma_start(b1t[:], b1[e])
            loaded[e] = (w1t, b1t)

        def _load_w2(e, after=None):
            w2t = w2pool.tile([128, KH * D], BF, tag="w2", name="w2t")
            _wdma(w2t[:], w2[e])
            loaded_w2[e] = w2t

        def _l1_half(w1t, b1t, hts, hf, sbt=None):
            # layer 1 for one token half: hts[:, mh, hf] = gelu(w1.T @ xT + b1)
            # scaled by the expert's per-token gate weight when sbt is given
            for mh in range(KH):
                ph = fpsum.tile([128, NF], FP, tag="ph", name="ph")
                for kd in range(KD):
                    nc.tensor.matmul(
                        ph[:],
                        w1t[:, kd * H + mh * 128:kd * H + (mh + 1) * 128],
                        xT[kd][:, hf * NF:(hf + 1) * NF],
                        start=(kd == 0),
                        stop=(kd == KD - 1),
                    )
                hsl = hts[:, mh * TOK + hf * NF:mh * TOK + (hf + 1) * NF]
                nc.scalar.activation(hsl, ph[:], AF.Gelu, bias=b1t[:, mh:mh + 1])
                if sbt is not None:
                    nc.vector.tensor_tensor(
                        hsl, hsl, sbt[:, hf * NF:(hf + 1) * NF], op=ALU.mult
                    )

        loaded_w2 = {}
        xlast = {}
        hts_pair = {}
        w2_pair = {}

        # ---- stage 1: x load + transpose + gating, with expert-0 layer 1
        # interleaved so the PE stream stays dense (HAM warm) ---------------
        with (
            tc.tile_pool(name="xg", bufs=1) as xg_pool,
            tc.tile_pool(name="gpsum", bufs=2, space="PSUM") as gpsum,
            tc.tile_pool(name="gtmp", bufs=3) as gtmp,
        ):
            xg = [
                xg_pool.tile([128, TOK], FP, tag=f"xg{d}", name=f"xg{d}")
                for d in range(KD)
            ]

            # x host-transposed. fp32 (gating) streams on the gpsimd ring in
            # gating-consumption order with w1[0] interleaved at its use
            # point; bf16 xT rides the otherwise-idle sync/scalar HW queues.
            xq = [(0, 256), (256, 512), (512, 768), (768, 1024)]
            for qi, (c0, c1) in enumerate(xq):
                qs = slice(c0, c1)
                for dd in range(KD):
                    di = nc.gpsimd.dma_start(
                        xg[dd][:, qs], xt32[dd * 128:(dd + 1) * 128, qs]
                    )
                    xlast[("f", qi)] = di.ins
            for half in range(2):
                hs = slice(half * 512, (half + 1) * 512)
                for dd in range(KD):
                    eng = nc.sync if dd % 2 == 0 else nc.scalar
                    di = eng.dma_start(
                        xT[dd][:, hs], xt16[dd * 128:(dd + 1) * 128, hs]
                    )
                    xlast[("b", half)] = di.ins

            def _tchunk(t):
                # gating for this token chunk (exact fp32)
                ts = slice(t * 128, (t + 1) * 128)
                pg = gpsum.tile([128, E], FP, tag="pg", name="pg")
                for d in range(KD):
                    nc.tensor.matmul(
                        pg[:],
                        xg[d][:, ts],
                        gw_sb[:, d * E:(d + 1) * E],
                        start=(d == 0),
                        stop=(d == KD - 1),
                    )
                m = gtmp.tile([128, 1], FP, tag="m", name="m")
                nc.vector.tensor_reduce(m[:], pg[:], axis=AX.X, op=ALU.max)
                nm = gtmp.tile([128, 1], FP, tag="nm", name="nm")
                nc.vector.tensor_scalar(nm[:], m[:], -1.0, None, op0=ALU.mult)
                ex = gtmp.tile([128, E], FP, tag="ex", name="ex")
                nc.scalar.activation(ex[:], pg[:], AF.Exp, bias=nm[:, 0:1])
                ssum = gtmp.tile([128, 1], FP, tag="ssum", name="ssum")
                nc.vector.tensor_reduce(ssum[:], ex[:], axis=AX.X, op=ALU.add)
                r = gtmp.tile([128, 1], FP, tag="r", name="r")
                nc.vector.reciprocal(r[:], ssum[:])
                g = gtmp.tile([128, E], FP, tag="g", name="g")
                nc.vector.tensor_scalar(g[:], ex[:], r[:, 0:1], None, op0=ALU.mult)
                # top-2: m1 = max, m2 = max after suppressing the argmax
                m1 = gtmp.tile([128, 1], FP, tag="m1", name="m1")
                nc.vector.tensor_reduce(m1[:], g[:], axis=AX.X, op=ALU.max)
                is1 = gtmp.tile([128, E], FP, tag="is1", name="is1")
                nc.vector.tensor_scalar(
                    is1[:], g[:], m1[:, 0:1], None, op0=ALU.is_ge
                )
                g2 = gtmp.tile([128, E], FP, tag="g2", name="g2")
                nc.vector.tensor_scalar(g2[:], is1[:], -2.0, None, op0=ALU.mult)
                nc.vector.tensor_tensor(g2[:], g2[:], g[:], op=ALU.add)
                m2 = gtmp.tile([128, 1], FP, tag="m2", name="m2")
                nc.vector.tensor_reduce(m2[:], g2[:], axis=AX.X, op=ALU.max)
                tokw = gtmp.tile([128, 1], FP, tag="tokw", name="tokw")
                nc.vector.tensor_tensor(tokw[:], m1[:], m2[:], op=ALU.add)
                sel = gtmp.tile([128, E], FP, tag="sel", name="sel")
                nc.vector.tensor_scalar(
                    sel[:], g[:], m2[:, 0:1], None, op0=ALU.is_ge
                )
                sw = gtmp.tile([128, E], FP, tag="sw", name="sw")
                nc.vector.tensor_scalar(
                    sw[:], sel[:], tokw[:, 0:1], None, op0=ALU.mult
                )
                # transpose S chunk [128, E] -> ST[:, t*128:+128] (bf16)
                pst = gpsum.tile([128, 128], FP, tag="pst", name="pst")
                nc.tensor.transpose(pst[0:E, :], sw[:], ident[:])
                nc.vector.tensor_copy(ST[:, ts], pst[0:E, :])

            # all 8 gating chunks run back-to-back (the ring streams fp32 x
            # in the same order); expert 0's layer 1 follows once w1[0]
            # lands on the ring right after the gating stream
            for t in range(TT):
                _tchunk(t)
            _load_w1(0, after=xlast[("f", 3)])
            hts0 = hpool.tile([128, KH * TOK], BF, tag="h", name="hts0")
            hts_pair[0] = hts0
            _l1_half(loaded[0][0], loaded[0][1], hts0, 0)
            _load_w1(1)
            _l1_half(loaded[0][0], loaded[0][1], hts0, 1)

        # ---- stage 2: per-expert FFN (bf16), expert-pair PSUM accum -------
        with (
            tc.tile_pool(name="bpsum", bufs=1, space="PSUM") as bpsum,
            tc.tile_pool(name="ypsum", bufs=4, space="PSUM") as ypsum,
        ):
            def _sbt_for(e):
                # expert's S row to partition 0, then broadcast to all 128
                # partitions via a K=1 ones-matmul
                ste = sbpool.tile([1, TOK], BF, tag="ste", name="ste")
                nc.sync.dma_start(ste[:], ST[e:e + 1, :])
                sbt = sbpool.tile([128, TOK], BF, tag="sb", name="sbt")
                for hf in range(NHF):
                    pb = bpsum.tile([128, NF], FP, tag="pb", name="pb")
                    nc.tensor.matmul(
                        pb[:],
                        ones_row[:],
                        ste[:, hf * NF:(hf + 1) * NF],
                        start=True,
                        stop=True,
                    )
                    nc.vector.tensor_copy(sbt[:, hf * NF:(hf + 1) * NF], pb[:])
                return sbt

            def _scale(hts, sbt):
                for mh in range(KH):
                    for hf in range(NHF):
                        hsl = hts[
                            :, mh * TOK + hf * NF:mh * TOK + (hf + 1) * NF
                        ]
                        nc.vector.tensor_tensor(
                            hsl, hsl, sbt[:, hf * NF:(hf + 1) * NF],
                            op=ALU.mult,
                        )

            for e in range(E):
                eo = e % 2
                # w2 of the CURRENT pair is consumed before the w1 of
                # expert e+2 -- emit in ring-consumption order
                if e == 0:
                    _load_w2(0)
                    _load_w2(1)
                elif e + 1 < E:
                    _load_w2(e + 1)
                if e + 2 < E:
                    _load_w1(e + 2)
                w1t, b1t = loaded.pop(e)
                sbt = _sbt_for(e)
                if e > 0:
                    hts = hpool.tile([128, KH * TOK], BF, tag="h", name="hts")
                    hts_pair[eo] = hts
                    _l1_half(w1t, b1t, hts, 0, sbt=sbt)
                    _l1_half(w1t, b1t, hts, 1, sbt=sbt)
                else:
                    _scale(hts_pair[eo], sbt)
                w2_pair[eo] = loaded_w2.pop(e)
                if eo == 0:
                    continue
                # layer 2 for the expert pair (e-1, e), PSUM-accumulated
                for md in range(KD):
                    for hf in range(NHF):
                        py = ypsum.tile([128, NF], FP, tag="py", name="py")
                        for po in (0, 1):
                            for kh in range(KH):
                                nc.tensor.matmul(
                                    py[:],
                                    w2_pair[po][
                                        :,
                                        kh * D + md * 128:kh * D + (md + 1) * 128,
                                    ],
                                    hts_pair[po][
                                        :,
                                        kh * TOK + hf * NF:kh * TOK + (hf + 1) * NF,
                                    ],
                                    start=(po == 0 and kh == 0),
                                    stop=(po == 1 and kh == KH - 1 and e != 1),
                                )
                        if e == 1:
                            # + sum_e S_e[tok] * b2[e, d] as a rank-8 matmul
                            nc.tensor.matmul(
                                py[:],
                                b2T[:, md * 128:(md + 1) * 128],
                                ST[:, hf * NF:(hf + 1) * NF],
                                start=False,
                                stop=True,
                            )
                        asl = acc[md][:, hf * NF:(hf + 1) * NF]
                        if e == 1:
                            nc.vector.tensor_copy(asl, py[:])
                        else:
                            nc.vector.tensor_tensor(asl, asl, py[:], op=ALU.add)
                        if e == E - 1 and hf == NHF - 1:
                            # final pair: stream this d-chunk out right away
                            nc.gpsimd.dma_start(
                                outT[md * 128:(md + 1) * 128, :],
                                acc[md][:, :],
                            )


_CACHED_NC = None


def _build():
    global _CACHED_NC
    if _CACHED_NC is not None:
        return _CACHED_NC
    nc = bass.Bass(
        "TRN2", target_bir_lowering=False, debug=False, num_devices=N_CORES
    )
    xt32 = nc.dram_tensor("xt32", [D, TOK], FP, kind="ExternalInput").ap()
    xt16 = nc.dram_tensor("xt16", [D, TOK], BF, kind="ExternalInput").ap()
    gw = nc.dram_tensor("gate_w", [D, E], FP, kind="ExternalInput").ap()
    w1 = nc.dram_tensor("w1", [E, 128, KD * H], BF, kind="ExternalInput").ap()
    b1 = nc.dram_tensor("b1", [E, 128, KH], FP, kind="ExternalInput").ap()
    w2 = nc.dram_tensor("w2", [E, 128, KH * D], BF, kind="ExternalInput").ap()
    b2 = nc.dram_tensor("b2", [E, D], FP, kind="ExternalInput").ap()
    outT = nc.dram_tensor("outT", [D, TOK], FP, kind="ExternalOutput").ap()
    with tile.TileContext(nc) as tc:
        _emit(tc, xt32, xt16, gw, w1, b1, w2, b2, outT)
    _legalize_sync_waits(nc)
    _CACHED_NC = nc
    return nc


def run(inputs, **spmd_kwargs):
    """Shard, run on 8 cores, unshard. Returns (out [B,S,D], BassKernelResults)."""
    nc = _build()
    xf = np.asarray(inputs["x"], dtype=np.float32).reshape(NTOK, D)
    import ml_dtypes
    shared = {
        k: np.ascontiguousarray(np.asarray(inputs[k], dtype=np.float32))
        for k in ("gate_w", "b2")
    }
    # b1 pre-laid to the SBUF image [E, 128, KH] (the on-device rearrange
    # was a 512-way 4-byte scatter DMA that congested the sync queue)
    shared["b1"] = np.ascontiguousarray(
        np.asarray(inputs["b1"], dtype=np.float32)
        .reshape(E, KH, 128).transpose(0, 2, 1)
    )
    shared["w1"] = np.ascontiguousarray(
        np.asarray(inputs["w1"], dtype=np.float32)
        .astype(ml_dtypes.bfloat16)
        .reshape(E, KD, 128, H).transpose(0, 2, 1, 3).reshape(E, 128, KD * H)
    )
    shared["w2"] = np.ascontiguousarray(
        np.asarray(inputs["w2"], dtype=np.float32)
        .astype(ml_dtypes.bfloat16)
        .reshape(E, KH, 128, D).transpose(0, 2, 1, 3).reshape(E, 128, KH * D)
    )
    import ml_dtypes as _mld
    in_maps = []
    for c in range(N_CORES):
        shT = np.ascontiguousarray(xf[c * TOK:(c + 1) * TOK].T)
        in_maps.append({
            "xt32": shT,
            "xt16": shT.astype(_mld.bfloat16),
            **shared,
        })
    res = run_bass_kernel_spmd(nc, in_maps, list(range(N_CORES)), **spmd_kwargs)
    out = np.concatenate(
        [res.results[c]["outT"].T for c in range(N_CORES)], axis=0
    )
    return out.reshape(B, S, D).astype(np.float32, copy=False), res


def kernel(**inputs):
    out, _ = run(inputs)
    return out

